# revision 3
# baseline (speedup 1.0000x reference)
"""Relational GCN layer (gnn_message_passing) on 8 TRN2 NeuronCores.

Math (per reference):
    agg[r] = segment_sum(vals[r][:,None] * inp[src[r]], dst[r], N)   # [N, F]
    out    = sum_r agg[r] @ W[r]                                     # [N, F]

Strategy: shard OUTPUT NODES (dst) across the 8 cores (6250 nodes each) --
no collective needed. Each core handles the ~100k edges (all 8 relations)
whose dst lands in its slice:

  1. dma_gather the edges' src rows from bf16 inp (256B rows) directly into
     the msgs tile. Gathers are spread over 4 SWDGE queues (4-5x throughput
     vs 1 queue; measured 8.4 ns/row @1q vs ~2 ns/row @4q), 512 rows/call.
     Pad gather lanes walk distinct spread-out rows -- duplicate-row
     descriptors serialize on one HBM channel (~2x whole-kernel cost).
  2. Aggregation via "segment matmul": per (dst-tile of 128 nodes, relation)
     group, S[edge_lane, node_col] = val_e * (dst_e == col), built host-side
     in bf16 and streamed via the HWDGE DMA path (overlaps the SWDGE
     gathers; on-device S construction via DVE tensor_scalar or gpsimd
     local_scatter measured slower).
     aggT_tile = msgs_chunk.T @ S_chunk accumulated in PSUM -> [f_in, n].
  3. out_tile[n, f_out] = sum_r aggT(t,r).T @ W[r], accumulated in PSUM.
     W-stage matmuls are software-pipelined one batch behind the agg
     matmuls so the in-order PE never stalls on the PSUM evacuations.
  4. PSUM->SBUF evacuations on DVE (130ns vs 400ns on Act).

SPMD constraint: one program for all 8 cores -> chunk layout padded to the
max across cores per (tile, relation, src-half) group. src indices must fit
in int16 for dma_gather -> edges split into src < 32768 ("lo") and rest
("hi", gathered with a base offset).

vs v2: 4 SWDGE queues (4-5x gather throughput), 512-row gather calls,
W-stage matmuls software-pipelined one batch behind the agg matmuls so the
in-order PE never stalls on Activation-engine PSUM evacuations, BATCH=64,
deeper PSUM/SBUF pools, no src sort (bench showed sorting hurts).
"""

import numpy as np
import ml_dtypes

N, R, E, F = 50000, 8, 100000, 128
C = 8
NPC = N // C
TILE = 128
T = -(-NPC // TILE)
HALF = 32768
BATCH = 64

F32 = np.float32
BF16 = ml_dtypes.bfloat16


def _build_layout(src, dst, vals):
    src = np.asarray(src)
    dst = np.asarray(dst)
    vals = np.asarray(vals)

    group_edges = [[[None] * R for _ in range(T)] for _ in range(C)]
    for r in range(R):
        d = dst[r]
        s = src[r]
        v = vals[r]
        order = np.argsort(d, kind="stable")
        ds = d[order]
        for c in range(C):
            a = np.searchsorted(ds, c * NPC, "left")
            b = np.searchsorted(ds, (c + 1) * NPC, "left")
            eidx = order[a:b]
            dl = ds[a:b] - c * NPC
            for t in range(T):
                ta = np.searchsorted(dl, t * TILE, "left")
                tb = np.searchsorted(dl, (t + 1) * TILE, "left")
                ge = eidx[ta:tb]
                gs = s[ge]
                gd = d[ge] - (c * NPC + t * TILE)
                gv = v[ge]
                lo = gs < HALF
                group_edges[c][t][r] = (
                    gs[lo], gd[lo], gv[lo], gs[~lo], gd[~lo], gv[~lo])

    n_lo = np.zeros((T, R), int)
    n_hi = np.zeros((T, R), int)
    for t in range(T):
        for r in range(R):
            n_lo[t, r] = max(-(-len(group_edges[c][t][r][0]) // 128) for c in range(C))
            n_hi[t, r] = max(-(-len(group_edges[c][t][r][3]) // 128) for c in range(C))

    groups = []
    for t in range(T):
        for r in range(R):
            if n_lo[t, r] + n_hi[t, r] > 0:
                groups.append((t, r, int(n_lo[t, r]), int(n_hi[t, r])))

    batches = []
    cur, cur_sz = [], 0
    for g in groups:
        sz = g[2] + g[3]
        if cur and cur_sz + sz > BATCH:
            batches.append(cur)
            cur, cur_sz = [], 0
        cur.append(g)
        cur_sz += sz
    if cur:
        batches.append(cur)

    meta_batches = []
    smat_cols = 0
    lo_cols = 0
    hi_cols = 0
    for bg in batches:
        blo = sum(g[2] for g in bg)
        bhi = sum(g[3] for g in bg)
        ginfo = []
        lo_off = 0
        hi_off = 0
        for (t, r, glo, ghi) in bg:
            ginfo.append(dict(t=t, r=r, nlo=glo, nhi=ghi,
                              lo_off=lo_off, hi_off=hi_off))
            lo_off += glo
            hi_off += ghi
        ntot = blo + bhi
        meta_batches.append(dict(
            groups=ginfo, n_lo=blo, n_hi=bhi,
            smat_base=smat_cols, lo_base=lo_cols, hi_base=hi_cols,
        ))
        smat_cols += ntot
        lo_cols += blo * 8
        hi_cols += bhi * 8

    tile_first = {}
    tile_last = {}
    gi = 0
    for bg in meta_batches:
        for g in bg["groups"]:
            t = g["t"]
            if t not in tile_first:
                tile_first[t] = gi
            tile_last[t] = gi
            gi += 1

    meta = dict(batches=meta_batches, nchunk=smat_cols,
                lo_cols=lo_cols, hi_cols=hi_cols,
                tile_first=tile_first, tile_last=tile_last)

    per_core = []
    for c in range(C):
        gidx_lo = np.zeros((16, max(lo_cols, 8)), np.int16)
        gidx_hi = np.zeros((16, max(hi_cols, 8)), np.int16)
        smat = np.zeros((128, max(smat_cols, 1) * 128), BF16)
        # pad gather lanes walk distinct rows (duplicate-row descriptors
        # serialize on one HBM channel; see gbench dup33)
        pad_ctr = [0, 0]
        for bg in meta_batches:
            for g in bg["groups"]:
                (ls, ld, lv, hs, hd, hv) = group_edges[c][g["t"]][g["r"]]
                for (es, ed, ev, nch, off, base, arr, sub) in (
                    (ls, ld, lv, g["nlo"], g["lo_off"], bg["lo_base"], gidx_lo, 0),
                    (hs, hd, hv, g["nhi"], g["hi_off"], bg["hi_base"], gidx_hi, HALF),
                ):
                    if nch == 0:
                        continue
                    ne = len(es)
                    vec = np.zeros(nch * 128, np.int16)
                    vec[:ne] = (es - sub).astype(np.int16)
                    npad = nch * 128 - ne
                    if npad:
                        half_i = 0 if sub == 0 else 1
                        rng_sz = HALF if sub == 0 else (N - HALF)
                        p0 = pad_ctr[half_i]
                        vec[ne:] = ((p0 + np.arange(npad)) % rng_sz).astype(np.int16)
                        pad_ctr[half_i] = (p0 + npad) % rng_sz
                    wr = vec.reshape(-1, 16).T
                    arr[:, base + off * 8: base + (off + nch) * 8] = wr
                    i = np.arange(ne)
                    slot = bg["smat_base"] + (0 if sub == 0 else bg["n_lo"]) \
                        + off + i // 128
                    smat[i % 128, slot * 128 + ed] = ev.astype(BF16)
        per_core.append(dict(
            gidx_lo=np.tile(gidx_lo, (8, 1)),
            gidx_hi=np.tile(gidx_hi, (8, 1)),
            smat=smat,
        ))
    return meta, per_core


def _build_program(meta, reps=1, nq=4, sp=True, gcall=4, scratch=16384, mbufs=3, sbufs=3, copy_eng='dve', iota_bf16=False):
    import concourse.bacc as bacc
    import concourse.mybir as mybir
    import concourse.tile as tile

    f32 = mybir.dt.float32
    bf16 = mybir.dt.bfloat16
    i16 = mybir.dt.int16
    EQ = mybir.AluOpType.is_equal
    MUL = mybir.AluOpType.mult
    COPY = mybir.ActivationFunctionType.Copy

    nc = bacc.Bacc(None, target_bir_lowering=False, num_swdge_queues=nq,
                   dynamic_dma_scratch_size=scratch)

    inp_d = nc.dram_tensor("inp", [N, F], bf16, kind="ExternalInput")
    gilo_d = nc.dram_tensor("gidx_lo", [128, max(meta["lo_cols"], 8)], i16,
                            kind="ExternalInput")
    gihi_d = nc.dram_tensor("gidx_hi", [128, max(meta["hi_cols"], 8)], i16,
                            kind="ExternalInput")
    smat_d = nc.dram_tensor("smat", [128, max(meta["nchunk"], 1) * 128], bf16,
                            kind="ExternalInput")
    w_d = nc.dram_tensor("weights", [R, F, F], f32, kind="ExternalInput")
    out_d = nc.dram_tensor("out", [NPC, F], f32, kind="ExternalOutput")

    with tile.TileContext(nc) as tc:
        with (
            tc.tile_pool(name="const", bufs=1) as cpool,
            tc.tile_pool(name="msgs", bufs=mbufs) as mpool,
            tc.tile_pool(name="stile", bufs=sbufs) as spool,
            tc.tile_pool(name="aggT", bufs=32) as apool,
            tc.tile_pool(name="osb", bufs=3) as opool,
            tc.tile_pool(name="psA", bufs=5, space="PSUM") as psum_a,
            tc.tile_pool(name="psO", bufs=3, space="PSUM") as psum_o,
        ):
            wtile = cpool.tile([128, R * F], f32)
            for r in range(R):
                nc.sync.dma_start(wtile[:, r * F:(r + 1) * F], w_d[r])
            gilo = cpool.tile([128, max(meta["lo_cols"], 8)], i16)
            nc.sync.dma_start(gilo[:], gilo_d[:])
            gihi = cpool.tile([128, max(meta["hi_cols"], 8)], i16)
            nc.sync.dma_start(gihi[:], gihi_d[:])


            gq = 0
            for _rep in range(reps):
                out_ps = {}
                pending = []   # deferred W-stage ops: (t, r, gidx, aggT tile)

                def flush_pending():
                    for (t, r, gidx, aggT) in pending:
                        if meta["tile_first"][t] == gidx:
                            out_ps[t] = psum_o.tile([128, F], f32, tag="ops",
                                                    name="ops")
                        nc.tensor.matmul(
                            out_ps[t][:], aggT[:], wtile[:, r * F:(r + 1) * F],
                            start=(meta["tile_first"][t] == gidx),
                            stop=(meta["tile_last"][t] == gidx),
                            skip_group_check=True,
                        )
                        if meta["tile_last"][t] == gidx:
                            osb = opool.tile([128, F], f32, tag="osb")
                            nc.scalar.activation(osb[:], out_ps[t][:], COPY)
                            rows = min(TILE, NPC - t * TILE)
                            nc.sync.dma_start(
                                out_d[t * TILE: t * TILE + rows, :],
                                osb[0:rows, :])
                            del out_ps[t]
                    pending.clear()

                gidx = 0
                for bg in meta["batches"]:
                    blo, bhi = bg["n_lo"], bg["n_hi"]
                    ntot = blo + bhi
                    msgs = mpool.tile([128, BATCH, F], bf16, tag="msgs")
                    for (nch, off, base, gi_t, src_ap) in (
                        (blo, 0, bg["lo_base"], gilo, inp_d[0:HALF, :]),
                        (bhi, blo, bg["hi_base"], gihi, inp_d[HALF:N, :]),
                    ):
                        for c0 in range(0, nch, gcall):
                            cn = min(gcall, nch - c0)
                            nc.gpsimd.dma_gather(
                                msgs[:, off + c0: off + c0 + cn, :], src_ap,
                                gi_t[:, base + c0 * 8: base + (c0 + cn) * 8],
                                cn * 128, cn * 128, F,
                                single_packet=sp,
                                queue_num=gq % nq,
                            )
                            gq += 1
                    stile = spool.tile([128, BATCH, F], bf16, tag="stile")
                    nc.sync.dma_start(
                        stile[:, 0:ntot, :],
                        smat_d[:, bg["smat_base"] * 128:
                               (bg["smat_base"] + ntot) * 128])

                    nxt = []
                    for g in bg["groups"]:
                        t, r = g["t"], g["r"]
                        chunks = [g["lo_off"] + k for k in range(g["nlo"])] + \
                                 [blo + g["hi_off"] + k for k in range(g["nhi"])]
                        agg = psum_a.tile([128, F], f32, tag="agg")
                        for ci, ch in enumerate(chunks):
                            nc.tensor.matmul(
                                agg[:], msgs[:, ch, :], stile[:, ch, :],
                                start=(ci == 0), stop=(ci == len(chunks) - 1),
                            )
                        aggT = apool.tile([128, F], f32, tag="aggT")
                        if copy_eng == "act" or (copy_eng == "mix" and gidx % 2):
                            nc.scalar.activation(aggT[:], agg[:], COPY)
                        else:
                            nc.vector.tensor_copy(aggT[:], agg[:])
                        nxt.append((t, r, gidx, aggT))
                        gidx += 1

                    # W-stage for the PREVIOUS batch (PE reaches these only
                    # after this batch's agg matmuls -> Act copies are done)
                    flush_pending()
                    pending.extend(nxt)
                flush_pending()

    nc.compile()
    return nc


def prepare(inputs):
    inp = np.asarray(inputs["inp"], F32).astype(BF16)
    weights = np.asarray(inputs["weights"], F32)
    meta, per_core = _build_layout(inputs["src"], inputs["dst"], inputs["vals"])
    in_maps = [
        dict(inp=inp, weights=weights, gidx_lo=pc["gidx_lo"],
             gidx_hi=pc["gidx_hi"], smat=pc["smat"])
        for pc in per_core
    ]
    return dict(meta=meta, in_maps=in_maps)


def build(prep, reps=1):
    return _build_program(prep["meta"], reps=reps)


def finish(prep, res):
    out = np.concatenate([res.results[c]["out"] for c in range(C)], axis=0)
    return out.astype(F32)


def kernel(inp, src, dst, vals, weights):
    from concourse.bass_utils import run_bass_kernel_spmd

    prep = prepare(dict(inp=inp, src=src, dst=dst, vals=vals, weights=weights))
    nc = build(prep)
    res = run_bass_kernel_spmd(nc, prep["in_maps"], core_ids=list(range(C)))
    return finish(prep, res)


# revision 5
# speedup vs baseline: 1.6600x; 1.6600x over previous
"""Relational GCN layer (gnn_message_passing) on 8 TRN2 NeuronCores.

Math (per reference):
    agg[r] = segment_sum(vals[r][:,None] * inp[src[r]], dst[r], N)   # [N, F]
    out    = sum_r agg[r] @ W[r]                                     # [N, F]

Strategy: shard OUTPUT NODES (dst) across the 8 cores (6250 nodes each) --
no collective needed. Each core handles the ~100k edges (all 8 relations)
whose dst lands in its slice:

  1. dma_gather the edges' src rows from bf16 inp (256B rows) directly into
     the msgs tile. Gathers are spread over 4 SWDGE queues (4-5x throughput
     vs 1 queue; measured 8.4 ns/row @1q vs ~2 ns/row @4q), 512 rows/call.
     Pad gather lanes walk distinct spread-out rows -- duplicate-row
     descriptors serialize on one HBM channel (~2x whole-kernel cost).
  2. Aggregation via "segment matmul": per (dst-tile of 128 nodes, relation)
     group, S[edge_lane, node_col] = val_e * (dst_e == col), built host-side
     in bf16 and streamed via the HWDGE DMA path (overlaps the SWDGE
     gathers; on-device S construction via DVE tensor_scalar or gpsimd
     local_scatter measured slower).
     aggT_tile = msgs_chunk.T @ S_chunk accumulated in PSUM -> [f_in, n].
  3. out_tile[n, f_out] = sum_r aggT(t,r).T @ W[r], accumulated in PSUM.
     W-stage matmuls are software-pipelined one batch behind the agg
     matmuls so the in-order PE never stalls on the PSUM evacuations.
  4. PSUM->SBUF evacuations on DVE (130ns vs 400ns on Act).

SPMD constraint: one program for all 8 cores -> chunk layout padded to the
max across cores per (tile, relation, src-half) group. src indices must fit
in int16 for dma_gather -> edges split into src < 32768 ("lo") and rest
("hi", gathered with a base offset).
"""

import numpy as np
import ml_dtypes

N, R, E, F = 50000, 8, 100000, 128
C = 8
NPC = N // C
TILE = 128
T = -(-NPC // TILE)
HALF = 32768
BATCH = 64

F32 = np.float32
BF16 = ml_dtypes.bfloat16


def _build_layout(src, dst, vals):
    src = np.asarray(src)
    dst = np.asarray(dst)
    vals = np.asarray(vals)

    group_edges = [[[None] * R for _ in range(T)] for _ in range(C)]
    for r in range(R):
        d = dst[r]
        s = src[r]
        v = vals[r]
        order = np.argsort(d, kind="stable")
        ds = d[order]
        for c in range(C):
            a = np.searchsorted(ds, c * NPC, "left")
            b = np.searchsorted(ds, (c + 1) * NPC, "left")
            eidx = order[a:b]
            dl = ds[a:b] - c * NPC
            for t in range(T):
                ta = np.searchsorted(dl, t * TILE, "left")
                tb = np.searchsorted(dl, (t + 1) * TILE, "left")
                ge = eidx[ta:tb]
                gs = s[ge]
                gd = d[ge] - (c * NPC + t * TILE)
                gv = v[ge]
                lo = gs < HALF
                group_edges[c][t][r] = (
                    gs[lo], gd[lo], gv[lo], gs[~lo], gd[~lo], gv[~lo])

    n_lo = np.zeros((T, R), int)
    n_hi = np.zeros((T, R), int)
    for t in range(T):
        for r in range(R):
            n_lo[t, r] = max(-(-len(group_edges[c][t][r][0]) // 128) for c in range(C))
            n_hi[t, r] = max(-(-len(group_edges[c][t][r][3]) // 128) for c in range(C))

    groups = []
    for t in range(T):
        for r in range(R):
            if n_lo[t, r] + n_hi[t, r] > 0:
                groups.append((t, r, int(n_lo[t, r]), int(n_hi[t, r])))

    batches = []
    cur, cur_sz = [], 0
    for g in groups:
        sz = g[2] + g[3]
        if cur and cur_sz + sz > BATCH:
            batches.append(cur)
            cur, cur_sz = [], 0
        cur.append(g)
        cur_sz += sz
    if cur:
        batches.append(cur)

    meta_batches = []
    smat_cols = 0
    lo_cols = 0
    hi_cols = 0
    for bg in batches:
        blo = sum(g[2] for g in bg)
        bhi = sum(g[3] for g in bg)
        ginfo = []
        lo_off = 0
        hi_off = 0
        for (t, r, glo, ghi) in bg:
            ginfo.append(dict(t=t, r=r, nlo=glo, nhi=ghi,
                              lo_off=lo_off, hi_off=hi_off))
            lo_off += glo
            hi_off += ghi
        ntot = blo + bhi
        meta_batches.append(dict(
            groups=ginfo, n_lo=blo, n_hi=bhi,
            smat_base=smat_cols, lo_base=lo_cols, hi_base=hi_cols,
        ))
        smat_cols += ntot
        lo_cols += blo * 8
        hi_cols += bhi * 8

    tile_first = {}
    tile_last = {}
    gi = 0
    for bg in meta_batches:
        for g in bg["groups"]:
            t = g["t"]
            if t not in tile_first:
                tile_first[t] = gi
            tile_last[t] = gi
            gi += 1

    meta = dict(batches=meta_batches, nchunk=smat_cols,
                lo_cols=lo_cols, hi_cols=hi_cols,
                tile_first=tile_first, tile_last=tile_last)

    per_core = []
    for c in range(C):
        gidx_lo = np.zeros((16, max(lo_cols, 8)), np.int16)
        gidx_hi = np.zeros((16, max(hi_cols, 8)), np.int16)
        smat = np.zeros((128, max(smat_cols, 1) * 128), BF16)
        # pad gather lanes walk distinct rows (duplicate-row descriptors
        # serialize on one HBM channel; see gbench dup33)
        pad_ctr = [0, 0]
        for bg in meta_batches:
            for g in bg["groups"]:
                (ls, ld, lv, hs, hd, hv) = group_edges[c][g["t"]][g["r"]]
                for (es, ed, ev, nch, off, base, arr, sub) in (
                    (ls, ld, lv, g["nlo"], g["lo_off"], bg["lo_base"], gidx_lo, 0),
                    (hs, hd, hv, g["nhi"], g["hi_off"], bg["hi_base"], gidx_hi, HALF),
                ):
                    if nch == 0:
                        continue
                    ne = len(es)
                    vec = np.zeros(nch * 128, np.int16)
                    vec[:ne] = (es - sub).astype(np.int16)
                    npad = nch * 128 - ne
                    if npad:
                        half_i = 0 if sub == 0 else 1
                        rng_sz = HALF if sub == 0 else (N - HALF)
                        p0 = pad_ctr[half_i]
                        vec[ne:] = ((p0 + np.arange(npad)) % rng_sz).astype(np.int16)
                        pad_ctr[half_i] = (p0 + npad) % rng_sz
                    wr = vec.reshape(-1, 16).T
                    arr[:, base + off * 8: base + (off + nch) * 8] = wr
                    i = np.arange(ne)
                    slot = bg["smat_base"] + (0 if sub == 0 else bg["n_lo"]) \
                        + off + i // 128
                    smat[i % 128, slot * 128 + ed] = ev.astype(BF16)
        per_core.append(dict(
            gidx_lo=np.tile(gidx_lo, (8, 1)),
            gidx_hi=np.tile(gidx_hi, (8, 1)),
            smat=smat,
        ))
    return meta, per_core


def _build_program(meta, reps=1, nq=4, sp=True, gcall=4, scratch=16384, mbufs=3, sbufs=3, copy_eng='dve', iota_bf16=False, smat_eng='sync'):
    import concourse.bacc as bacc
    import concourse.mybir as mybir
    import concourse.tile as tile

    f32 = mybir.dt.float32
    bf16 = mybir.dt.bfloat16
    i16 = mybir.dt.int16
    EQ = mybir.AluOpType.is_equal
    MUL = mybir.AluOpType.mult
    COPY = mybir.ActivationFunctionType.Copy

    nc = bacc.Bacc(None, target_bir_lowering=False, num_swdge_queues=nq,
                   dynamic_dma_scratch_size=scratch)

    inp_d = nc.dram_tensor("inp", [N, F], bf16, kind="ExternalInput")
    gilo_d = nc.dram_tensor("gidx_lo", [128, max(meta["lo_cols"], 8)], i16,
                            kind="ExternalInput")
    gihi_d = nc.dram_tensor("gidx_hi", [128, max(meta["hi_cols"], 8)], i16,
                            kind="ExternalInput")
    smat_d = nc.dram_tensor("smat", [128, max(meta["nchunk"], 1) * 128], bf16,
                            kind="ExternalInput")
    w_d = nc.dram_tensor("weights", [R, F, F], f32, kind="ExternalInput")
    out_d = nc.dram_tensor("out", [NPC, F], f32, kind="ExternalOutput")

    with tile.TileContext(nc) as tc:
        with (
            tc.tile_pool(name="const", bufs=1) as cpool,
            tc.tile_pool(name="msgs", bufs=mbufs) as mpool,
            tc.tile_pool(name="stile", bufs=sbufs) as spool,
            tc.tile_pool(name="aggT", bufs=32) as apool,
            tc.tile_pool(name="osb", bufs=3) as opool,
            tc.tile_pool(name="psA", bufs=5, space="PSUM") as psum_a,
            tc.tile_pool(name="psO", bufs=3, space="PSUM") as psum_o,
        ):
            wtile = cpool.tile([128, R * F], f32)
            for r in range(R):
                nc.sync.dma_start(wtile[:, r * F:(r + 1) * F], w_d[r])
            gilo = cpool.tile([128, max(meta["lo_cols"], 8)], i16)
            nc.sync.dma_start(gilo[:], gilo_d[:])
            gihi = cpool.tile([128, max(meta["hi_cols"], 8)], i16)
            nc.sync.dma_start(gihi[:], gihi_d[:])


            gq = 0
            for _rep in range(reps):
                out_ps = {}
                pending = []   # deferred W-stage ops: (t, r, gidx, aggT tile)

                def flush_pending():
                    for (t, r, gidx, aggT) in pending:
                        if meta["tile_first"][t] == gidx:
                            out_ps[t] = psum_o.tile([128, F], f32, tag="ops",
                                                    name="ops")
                        nc.tensor.matmul(
                            out_ps[t][:], aggT[:], wtile[:, r * F:(r + 1) * F],
                            start=(meta["tile_first"][t] == gidx),
                            stop=(meta["tile_last"][t] == gidx),
                            skip_group_check=True,
                        )
                        if meta["tile_last"][t] == gidx:
                            osb = opool.tile([128, F], f32, tag="osb")
                            nc.scalar.activation(osb[:], out_ps[t][:], COPY)
                            rows = min(TILE, NPC - t * TILE)
                            nc.sync.dma_start(
                                out_d[t * TILE: t * TILE + rows, :],
                                osb[0:rows, :])
                            del out_ps[t]
                    pending.clear()

                gidx = 0
                for bg in meta["batches"]:
                    blo, bhi = bg["n_lo"], bg["n_hi"]
                    ntot = blo + bhi
                    msgs = mpool.tile([128, BATCH, F], bf16, tag="msgs")
                    for (nch, off, base, gi_t, src_ap) in (
                        (blo, 0, bg["lo_base"], gilo, inp_d[0:HALF, :]),
                        (bhi, blo, bg["hi_base"], gihi, inp_d[HALF:N, :]),
                    ):
                        for c0 in range(0, nch, gcall):
                            cn = min(gcall, nch - c0)
                            nc.gpsimd.dma_gather(
                                msgs[:, off + c0: off + c0 + cn, :], src_ap,
                                gi_t[:, base + c0 * 8: base + (c0 + cn) * 8],
                                cn * 128, cn * 128, F,
                                single_packet=sp,
                                queue_num=gq % nq,
                            )
                            gq += 1
                    stile = spool.tile([128, BATCH, F], bf16, tag="stile")
                    getattr(nc, smat_eng).dma_start(
                        stile[:, 0:ntot, :],
                        smat_d[:, bg["smat_base"] * 128:
                               (bg["smat_base"] + ntot) * 128])

                    nxt = []
                    for g in bg["groups"]:
                        t, r = g["t"], g["r"]
                        chunks = [g["lo_off"] + k for k in range(g["nlo"])] + \
                                 [blo + g["hi_off"] + k for k in range(g["nhi"])]
                        agg = psum_a.tile([128, F], f32, tag="agg")
                        for ci, ch in enumerate(chunks):
                            nc.tensor.matmul(
                                agg[:], msgs[:, ch, :], stile[:, ch, :],
                                start=(ci == 0), stop=(ci == len(chunks) - 1),
                            )
                        aggT = apool.tile([128, F], f32, tag="aggT")
                        if copy_eng == "act" or (copy_eng == "mix" and gidx % 2):
                            nc.scalar.activation(aggT[:], agg[:], COPY)
                        else:
                            nc.vector.tensor_copy(aggT[:], agg[:])
                        nxt.append((t, r, gidx, aggT))
                        gidx += 1

                    # W-stage for the PREVIOUS batch (PE reaches these only
                    # after this batch's agg matmuls -> Act copies are done)
                    flush_pending()
                    pending.extend(nxt)
                flush_pending()

    nc.compile()
    return nc


def prepare(inputs):
    inp = np.asarray(inputs["inp"], F32).astype(BF16)
    weights = np.asarray(inputs["weights"], F32)
    meta, per_core = _build_layout(inputs["src"], inputs["dst"], inputs["vals"])
    in_maps = [
        dict(inp=inp, weights=weights, gidx_lo=pc["gidx_lo"],
             gidx_hi=pc["gidx_hi"], smat=pc["smat"])
        for pc in per_core
    ]
    return dict(meta=meta, in_maps=in_maps)


def build(prep, reps=1):
    return _build_program(prep["meta"], reps=reps)


def finish(prep, res):
    out = np.concatenate([res.results[c]["out"] for c in range(C)], axis=0)
    return out.astype(F32)


def kernel(inp, src, dst, vals, weights):
    from concourse.bass_utils import run_bass_kernel_spmd

    prep = prepare(dict(inp=inp, src=src, dst=dst, vals=vals, weights=weights))
    nc = build(prep)
    res = run_bass_kernel_spmd(nc, prep["in_maps"], core_ids=list(range(C)))
    return finish(prep, res)
